# revision 6
# baseline (speedup 1.0000x reference)
"""Dilated self-attention TRN2 Bass kernel.

Problem (hardcoded): B=2, N=8192, C=256, WS=[2048,4096,8192], RS=[1,2,4],
HEAD_IDX=0 -> G=7 groups of s=2048 rows each.

Sharding: 8 cores, core d = (b=d//4, q=d%4) owns output positions
[2048q, 2048(q+1)) of batch b.  Each core computes, fully locally:
  job A: the r=1 segment group g_q of its quarter (2048 queries, causal)
  job B: the 1024-query half of the r=2 group whose outputs land in its quarter
  job C: the 512-query quarter of the r=4 group landing in its quarter
Attention is computed unnormalized: U = exp(scores/16) @ [V | 1], so the last
column carries the softmax denominator.  The cross-group scatter-add combine is
local: U_B rows scatter into the quarter at stride 2, U_C at stride 4 (done via
SWDGE DMA accumulate into a resident SBUF accumulator), then out = U[:, :256]
/ U[:, 256].  Host does only gather/transpose/zero-pad (sharding) and
concatenation (unsharding) - no arithmetic.

The SPMD program is identical on all cores; per-core differences are carried by
input data only (pre-gathered transposed slabs + bias vectors that disable
padded prefix k-tiles via exp's bias = -1e9).
"""

import numpy as np

B, N, C = 2, 8192, 256
S = 2048          # rows per group
NCORES = 8
SCALE = 0.0625    # 1/sqrt(256)
NEG = -1.0e9

_PROG = None      # cached compiled Bass program


def _build_program(mm_fp32=False):
    import concourse.mybir as mybir
    import concourse.tile as tile
    from concourse import bacc

    F32 = mybir.dt.float32
    MMDT = mybir.dt.float32 if mm_fp32 else mybir.dt.float32r
    Exp = mybir.ActivationFunctionType.Exp

    nc = bacc.Bacc("TRN2", target_bir_lowering=False, debug=False,
                   num_devices=NCORES)

    xA = nc.dram_tensor("xA", [C, S], MMDT, kind="ExternalInput")
    xB = nc.dram_tensor("xB", [C, S], MMDT, kind="ExternalInput")
    xC = nc.dram_tensor("xC", [C, S], MMDT, kind="ExternalInput")
    wq_d = nc.dram_tensor("wq", [C, C], MMDT, kind="ExternalInput")
    wk_d = nc.dram_tensor("wk", [C, C], MMDT, kind="ExternalInput")
    wv_d = nc.dram_tensor("wv", [C, C], MMDT, kind="ExternalInput")
    biasB_d = nc.dram_tensor("biasB", [128, 8], F32, kind="ExternalInput")
    biasC_d = nc.dram_tensor("biasC", [128, 12], F32, kind="ExternalInput")
    out_d = nc.dram_tensor("out", [S, C], F32, kind="ExternalOutput")

    # job specs: (x dram, n q rows, q row offset in slab, prefix k-tiles,
    #             bias dram or None, scatter stride)
    jobs = [
        dict(x=xA, nq=2048, q0=0, P=0, bias=None, stride=1),
        dict(x=xB, nq=1024, q0=1024, P=8, bias=biasB_d, stride=2),
        dict(x=xC, nq=512, q0=1536, P=12, bias=biasC_d, stride=4),
    ]

    with tile.TileContext(nc) as tc:
        with (
            tc.tile_pool(name="const", bufs=1) as cpool,
            tc.tile_pool(name="xsb", bufs=3) as xpool,
            tc.tile_pool(name="ktp", bufs=3) as ktpool,
            tc.tile_pool(name="qtp", bufs=2) as qtpool,
            tc.tile_pool(name="vext", bufs=18) as vpool,
            tc.tile_pool(name="probs", bufs=18) as ppool,
            tc.tile_pool(name="stage", bufs=4) as spool,
            tc.tile_pool(name="acc", bufs=1) as apool,
            tc.tile_pool(name="fin", bufs=4) as fpool,
            tc.tile_pool(name="ps_s", bufs=2, space="PSUM") as ps_scores,
            tc.tile_pool(name="ps_u", bufs=4, space="PSUM") as ps_u,
            tc.tile_pool(name="ps_p", bufs=2, space="PSUM") as ps_proj,
        ):
            # ---- constants ----
            w_sb = {}
            for nm, dram in (("q", wq_d), ("k", wk_d), ("v", wv_d)):
                for ci in range(2):
                    t = cpool.tile([128, 256], MMDT, tag=f"w{nm}{ci}")
                    nc.sync.dma_start(t[:], dram[128 * ci:128 * (ci + 1), :])
                    w_sb[nm, ci] = t
            biasB_t = cpool.tile([128, 8], F32, tag="biasB")
            nc.sync.dma_start(biasB_t[:], biasB_d[:])
            biasC_t = cpool.tile([128, 12], F32, tag="biasC")
            nc.sync.dma_start(biasC_t[:], biasC_d[:])
            bias_sb = {id(biasB_d): biasB_t, id(biasC_d): biasC_t}

            ones_t = cpool.tile([128, 512], F32, tag="ones")
            nc.gpsimd.memset(ones_t[:], 1.0)
            ones01 = cpool.tile([128, 2], F32, tag="ones01")
            nc.vector.memset(ones01[:, 0:1], 1.0)
            nc.vector.memset(ones01[:, 1:2], 0.0)
            masks = []
            for j in range(4):
                mf = cpool.tile([128, 512], F32, tag=f"maskf{j}", name=f"maskf{j}")
                nc.gpsimd.affine_select(
                    out=mf[:], in_=ones_t[:],
                    compare_op=mybir.AluOpType.is_ge,
                    fill=0.0, base=-128 * j,
                    pattern=[[1, 512]], channel_multiplier=-1,
                )
                m = cpool.tile([128, 512], MMDT, tag=f"mask{j}", name=f"mask{j}")
                nc.vector.tensor_copy(m[:], mf[:])
                masks.append(m)

            # persistent accumulator: 16 tiles of [128, 257] covering the
            # 2048 output positions of this core's quarter
            acc = [apool.tile([128, 257], F32, tag=f"acc{t}", name=f"acc{t}")
                   for t in range(16)]

            # ---- jobs ----
            for jn, job in enumerate(jobs):
                nq, q0, P = job["nq"], job["q0"], job["P"]
                stride = job["stride"]
                nkt_all = 16          # k/v tiles per job (always full slab)

                xsb = []
                for ci in range(2):
                    t = xpool.tile([128, S], MMDT, tag="xsb")
                    nc.sync.dma_start(t[:], job["x"][128 * ci:128 * (ci + 1), :])
                    xsb.append(t)

                # K^T [c, k] as 2 c-tiles of [128, 2048]
                kt_sb = [ktpool.tile([128, S], MMDT, tag="kt", name=f"kt{jn}_{_i}")
                         for _i in range(2)]
                for co in range(2):
                    for kc in range(4):
                        ps = ps_proj.tile([128, 512], F32, tag="proj")
                        for ci in range(2):
                            nc.tensor.matmul(
                                ps[:], w_sb["k", ci][:, 128 * co:128 * (co + 1)],
                                xsb[ci][:, 512 * kc:512 * (kc + 1)],
                                start=(ci == 0), stop=(ci == 1))
                        nc.vector.tensor_copy(
                            kt_sb[co][:, 512 * kc:512 * (kc + 1)], ps[:])

                # Q^T [c, q] (only the query rows)
                qt_sb = [qtpool.tile([128, nq], MMDT, tag="qt", name=f"qt{jn}_{_i}")
                         for _i in range(2)]
                for co in range(2):
                    for qc in range(nq // 512):
                        ps = ps_proj.tile([128, 512], F32, tag="proj")
                        for ci in range(2):
                            nc.tensor.matmul(
                                ps[:], w_sb["q", ci][:, 128 * co:128 * (co + 1)],
                                xsb[ci][:, q0 + 512 * qc:q0 + 512 * (qc + 1)],
                                start=(ci == 0), stop=(ci == 1))
                        nc.vector.tensor_copy(
                            qt_sb[co][:, 512 * qc:512 * (qc + 1)], ps[:])

                # V [k, c] per k-tile, augmented with a ones column
                vext = []
                for kt in range(nkt_all):
                    ps = ps_proj.tile([128, 256], F32, tag="proj", name="psv")
                    for ci in range(2):
                        nc.tensor.matmul(
                            ps[:], xsb[ci][:, 128 * kt:128 * (kt + 1)],
                            w_sb["v", ci][:],
                            start=(ci == 0), stop=(ci == 1))
                    v = vpool.tile([128, 258], MMDT, tag="vext")
                    nc.vector.tensor_copy(v[:, 0:256], ps[:])
                    nc.vector.tensor_copy(v[:, 256:258], ones01[:])
                    vext.append(v)

                # ---- attention over 512-wide q blocks ----
                for i in range(nq // 512):
                    nkt = P + 4 * i + 4
                    probs = []
                    for kt in range(nkt):
                        ps = ps_scores.tile([128, 512], F32, tag="scores")
                        for ci in range(2):
                            nc.tensor.matmul(
                                ps[:], kt_sb[ci][:, 128 * kt:128 * (kt + 1)],
                                qt_sb[ci][:, 512 * i:512 * (i + 1)],
                                start=(ci == 0), stop=(ci == 1))
                        pb = ppool.tile([128, 512], MMDT, tag="probs")
                        if kt < P:
                            bias_ap = bias_sb[id(job["bias"])][:, kt:kt + 1]
                        else:
                            bias_ap = 0.0
                        nc.scalar.activation(pb[:], ps[:], Exp,
                                             bias=bias_ap, scale=SCALE)
                        jd = kt - (P + 4 * i)
                        if jd >= 0:
                            nc.vector.tensor_mul(pb[:], pb[:], masks[jd][:])
                        probs.append(pb)

                    for j in range(4):
                        nk = P + 4 * i + j + 1
                        ups = ps_u.tile([128, 258], F32, tag="u")
                        for kk in range(nk):
                            nc.tensor.matmul(
                                ups[:], probs[kk][:, 128 * j:128 * (j + 1)],
                                vext[kk][:],
                                start=(kk == 0), stop=(kk == nk - 1))
                        t_local = 4 * i + j  # q tile index within job
                        if stride == 1:
                            nc.vector.tensor_copy(acc[t_local][:], ups[:, 0:257])
                        elif stride == 2:
                            st = spool.tile([128, 257], F32, tag="stage")
                            nc.vector.tensor_copy(st[:], ups[:, 0:257])
                            for u in range(2):
                                nc.gpsimd.dma_start(
                                    acc[2 * t_local + u][0:128:2, :],
                                    st[64 * u:64 * (u + 1), :],
                                    accum_op=mybir.AluOpType.add)
                        else:
                            st = spool.tile([128, 257], F32, tag="stage")
                            nc.vector.tensor_copy(st[:], ups[:, 0:257])
                            for u in range(4):
                                nc.gpsimd.dma_start(
                                    acc[4 * t_local + u][0:128:4, :],
                                    st[32 * u:32 * (u + 1), :],
                                    accum_op=mybir.AluOpType.add)

            # ---- finalize: divide by denominator, store ----
            for t in range(16):
                rec = fpool.tile([128, 1], F32, tag="rec")
                nc.vector.reciprocal(rec[:], acc[t][:, 256:257])
                ot = fpool.tile([128, 256], F32, tag="fin")
                nc.vector.tensor_scalar_mul(ot[:], acc[t][:, 0:256], rec[:])
                nc.sync.dma_start(out_d[128 * t:128 * (t + 1), :], ot[:])

    nc.compile()
    return nc


def _get_program():
    global _PROG
    if _PROG is None:
        _PROG = _build_program()
    return _PROG


def make_in_maps(x, Wq, Wk, Wv):
    """Host-side sharding: pure gather / transpose / zero-pad, no arithmetic."""
    x = np.asarray(x, dtype=np.float32)
    Wq = np.ascontiguousarray(np.asarray(Wq, dtype=np.float32))
    Wk = np.ascontiguousarray(np.asarray(Wk, dtype=np.float32))
    Wv = np.ascontiguousarray(np.asarray(Wv, dtype=np.float32))
    in_maps = []
    for d in range(NCORES):
        b, q = divmod(d, 4)
        xA = np.ascontiguousarray(x[b, 2048 * q:2048 * (q + 1), :].T)

        seg = 0 if q < 2 else 4096
        grp2 = x[b, seg:seg + 4096:2, :]          # [2048, 256]
        r0 = 1024 * (q % 2)
        if r0 == 1024:
            rowsB = grp2                           # prefix real + diag
        else:
            rowsB = np.concatenate(
                [np.zeros((1024, C), np.float32), grp2[0:1024]], axis=0)
        xB = np.ascontiguousarray(rowsB.T)

        grp4 = x[b, 0:8192:4, :]                  # [2048, 256]
        r0c = 512 * q
        rowsC = np.concatenate(
            [grp4[0:r0c], np.zeros((1536 - r0c, C), np.float32),
             grp4[r0c:r0c + 512]], axis=0)
        xC = np.ascontiguousarray(rowsC.T)

        biasB = np.full((128, 8), 0.0 if r0 == 1024 else NEG, np.float32)
        biasC = np.zeros((128, 12), np.float32)
        biasC[:, 4 * q:] = NEG

        in_maps.append({
            "xA": xA, "xB": xB, "xC": xC,
            "wq": Wq, "wk": Wk, "wv": Wv,
            "biasB": biasB, "biasC": biasC,
        })
    return in_maps


def kernel(x, Wq, Wk, Wv):
    from concourse.bass_utils import run_bass_kernel_spmd

    nc = _get_program()
    in_maps = make_in_maps(x, Wq, Wk, Wv)
    res = run_bass_kernel_spmd(nc, in_maps, core_ids=list(range(NCORES)))
    out = np.empty((B, N, C), np.float32)
    for d in range(NCORES):
        b, q = divmod(d, 4)
        out[b, 2048 * q:2048 * (q + 1), :] = res.results[d]["out"]
    return out
